# revision 1
# baseline (speedup 1.0000x reference)
"""Trainium2 Bass kernel for nn_Encoder_P: unwrap-diff-square front-end + 4 dilated
convs with dense concatenation, fused end-to-end on-chip.

Strategy (pure data parallel, 1 batch sample per NeuronCore, 8 cores):
  - The unwrap/diff/pad chain collapses: cumsum cancels in the diff, so
    sq[h] = wrap(p[h] - p[h-1])^2 (row 0 = 0), wrap(v) = v - 2*pi*k with
    k = (v>=pi) + (v>=3pi) - (v<=-pi) - (v<=-3pi).
  - Duplicate concat channels are folded into effective conv weights
    (conv3: 8->7 input planes, conv4: 20->15).
  - Each conv runs on TensorE as banded matmuls over the H (partition) axis:
    lhsT is a banded [128,128] H-shift matrix built on-device (DVE) from 5
    shared shifted-identity masters scaled by runtime weight scalars; rhs is
    the input plane tile [128 H, 516 Wpad]; PSUM accumulates over (ci, kw).
  - Planes are stored as 5 overlapping H-tiles (stride 104, halo 12) of
    [128, 516] with zeroed W margins, so conv H/W reach never crosses a tile.
"""

import numpy as np

import concourse.bacc as bacc
import concourse.bass as bass
import concourse.mybir as mybir
import concourse.tile as tile
from concourse import bass_utils

F32 = mybir.dt.float32
MM_DT = mybir.dt.float32r  # full-rate fp32 matmul path (1 cyc/row at N>=256)
DEFAULT_MM = "f32r"  # flip to "bf16" only with HW-validated accuracy+speed

H = 512
W = 512
S = 107          # tile stride in rows (chosen so 512-(S*4-HALO) == 96, a legal
                 # compute-op partition start for the bottom edge-zero memset)
HALO = 12        # halo rows above/below each tile
NT = 5           # number of H tiles
WPAD = 516       # 2 zero cols + 512 + 2 zero cols
P = 128
PI = float(np.pi)

# conv specs: (dil, pad_top, pad_left, KH, KW)
CONV_GEOM = [
    (1, 1, 1, 4, 4),   # conv1: 4x4 dil1, 'same' pad (1,2)
    (2, 2, 2, 3, 3),   # conv2: 3x3 dil2, pad (2,2)
    (3, 1, 1, 2, 2),   # conv3: 2x2 dil3, pad (1,2)
    (4, 0, 0, 1, 1),   # conv4: 1x1
]

PLANE_NAMES = (
    ["sq", "c1_0", "c1_1"]
    + [f"c2_{i}" for i in range(4)]
    + [f"c3_{i}" for i in range(8)]
)
CONV_INPUTS = [
    ["sq"],
    ["c1_0", "c1_1", "sq"],
    [f"c2_{i}" for i in range(4)] + ["c1_0", "c1_1", "sq"],
    [f"c3_{i}" for i in range(8)] + [f"c2_{i}" for i in range(4)]
    + ["c1_0", "c1_1", "sq"],
]
CONV_OUT = [2, 4, 8, 16]
DELTAS = [-2, -1, 0, 1, 2]  # identity master shifts

# output channel -> source plane ("c4_o" channels handled separately)
CH_MAP = (
    [f"c4_{i}" for i in range(16)]
    + [f"c3_{i}" for i in range(8)]
    + [f"c2_{i}" for i in range(4)]
    + ["c1_0", "c1_1", "sq", "sq", "c1_0", "c1_1", "sq", "sq"]
    + [f"c2_{i}" for i in range(4)]
    + ["c1_0", "c1_1", "sq", "sq"]
    + ["c1_0", "c1_1", "sq", "sq"]
)

NSCAL = sum(
    CONV_OUT[c] * len(CONV_INPUTS[c]) * CONV_GEOM[c][3] * CONV_GEOM[c][4]
    for c in range(4)
)  # 604


def _fold_weights(w1, w2, w3, w4):
    w3f = np.zeros((8, 7, 2, 2), np.float32)
    w3f[:, :6] = w3[:, :6]
    w3f[:, 6] = w3[:, 6] + w3[:, 7]
    w4f = np.zeros((16, 15, 1, 1), np.float32)
    w4f[:, :12] = w4[:, :12]
    w4f[:, 12] = w4[:, 12] + w4[:, 16]
    w4f[:, 13] = w4[:, 13] + w4[:, 17]
    w4f[:, 14] = w4[:, 14] + w4[:, 15] + w4[:, 18] + w4[:, 19]
    return [w1.astype(np.float32), w2.astype(np.float32), w3f, w4f]


def _host_tables(inputs):
    """wtab [128, NSCAL], ident [5*128, 128], bias [128, 30] host arrays."""
    wf = _fold_weights(inputs["w1"], inputs["w2"], inputs["w3"], inputs["w4"])
    scal = []
    for c in range(4):
        dil, pad_top, _, KH, KW = CONV_GEOM[c]
        for o in range(CONV_OUT[c]):
            for ci in range(len(CONV_INPUTS[c])):
                for kw in range(KW):
                    for kh in range(KH):
                        scal.append(wf[c][o, ci, kh, kw])
    assert len(scal) == NSCAL
    wtab = np.tile(np.asarray(scal, np.float32)[None, :], (P, 1))
    ident = np.concatenate(
        [np.eye(P, dtype=np.float32, k=-d) for d in DELTAS], axis=0
    )
    bias = np.concatenate(
        [inputs["b1"], inputs["b2"], inputs["b3"], inputs["b4"]]
    ).astype(np.float32)
    bias = np.tile(bias[None, :], (P, 1))
    return wtab, ident, bias


def build_nc(loop_k=1, out_mode='full', skip_bands=False, mm='f32r'):
    nc = bacc.Bacc("TRN2", target_bir_lowering=False, debug=False)
    mm_dt = mybir.dt.bfloat16 if mm == 'bf16' else MM_DT

    def msafe(ap):
        # memset target: walrus rejects float32r memsets; bitcast those to f32
        return ap.bitcast(F32) if mm != 'bf16' else ap

    p_dram = nc.dram_tensor("p", [H, W], F32, kind="ExternalInput")
    ident_dram = nc.dram_tensor("ident", [5 * P, P], F32, kind="ExternalInput")
    wtab_dram = nc.dram_tensor("wtab", [P, NSCAL], F32, kind="ExternalInput")
    bias_dram = nc.dram_tensor("bias", [P, 30], F32, kind="ExternalInput")
    out_dram = nc.dram_tensor("out", [48, H, W], F32, kind="ExternalOutput")

    planes = {
        nm: nc.alloc_sbuf_tensor(f"pl_{nm}", [P, NT * WPAD], mm_dt)
        for nm in PLANE_NAMES
    }
    ident_sb = nc.alloc_sbuf_tensor("ident_sb", [P, 5 * P], F32)
    wtab_sb = nc.alloc_sbuf_tensor("wtab_sb", [P, NSCAL], F32)
    bias_sb = nc.alloc_sbuf_tensor("bias_sb", [P, 30], F32)

    def pslice(nm, t, c0, c1):
        return planes[nm][:, t * WPAD + c0 : t * WPAD + c1]

    with tile.TileContext(nc) as tc:
        with (
            tc.tile_pool(name="io", bufs=3) as io_pool,
            tc.tile_pool(name="front", bufs=2) as fr_pool,
            tc.tile_pool(name="bands", bufs=12) as band_pool,
            tc.tile_pool(name="psum", bufs=8, space="PSUM") as psum_pool,
            tc.tile_pool(name="c4st", bufs=3) as c4_pool,
        ):
            for _it in range(loop_k):
                # ---- parameter loads ----
                for j in range(5):
                    nc.sync.dma_start(
                        out=ident_sb[:, j * P : (j + 1) * P],
                        in_=ident_dram[j * P : (j + 1) * P, :],
                    )
                nc.sync.dma_start(out=wtab_sb[:], in_=wtab_dram[:])
                nc.sync.dma_start(out=bias_sb[:], in_=bias_dram[:])

                # ---- zero W margins of all planes (written once) ----
                for nm in PLANE_NAMES:
                    for t in range(NT):
                        nc.gpsimd.memset(msafe(pslice(nm, t, 0, 2)), 0.0)
                        nc.gpsimd.memset(msafe(pslice(nm, t, 514, 516)), 0.0)

                # ---- front-end: sq ----
                # A/B garbage regions are pre-zeroed so the out-of-image rows
                # compute v=0 -> sq=0, which is exactly the reference's zero pad.
                for t in range(NT):
                    p_lo = HALO if t == 0 else 0
                    p_hi = H - (S * (NT - 1) - HALO) if t == NT - 1 else P  # 96 at t=4
                    n = p_hi - p_lo
                    r_lo = S * t - HALO + p_lo
                    A = io_pool.tile([P, W], F32, tag="A")
                    B = io_pool.tile([P, W], F32, tag="B")
                    if t == 0:
                        nc.gpsimd.memset(A[0:32, :], 0.0)
                        nc.gpsimd.memset(B[0:32, :], 0.0)
                    if t == NT - 1:
                        nc.gpsimd.memset(A[96:P, :], 0.0)
                        nc.gpsimd.memset(B[96:P, :], 0.0)
                    nc.sync.dma_start(out=A[p_lo:p_hi, :], in_=p_dram[r_lo : r_lo + n, :])
                    if t == 0:
                        nc.sync.dma_start(
                            out=B[p_lo + 1 : p_hi, :], in_=p_dram[0 : n - 1, :]
                        )
                        nc.sync.dma_start(out=B[p_lo : p_lo + 1, :], in_=p_dram[0:1, :])
                    else:
                        nc.sync.dma_start(
                            out=B[p_lo:p_hi, :], in_=p_dram[r_lo - 1 : r_lo - 1 + n, :]
                        )
                    V = fr_pool.tile([P, W], F32, tag="V")
                    K1 = fr_pool.tile([P, W], F32, tag="K1")
                    K2 = fr_pool.tile([P, W], F32, tag="K2")
                    K3 = fr_pool.tile([P, W], F32, tag="K3")
                    K4 = fr_pool.tile([P, W], F32, tag="K4")
                    ao = mybir.AluOpType
                    nc.vector.tensor_tensor(V[:], A[:], B[:], ao.subtract)
                    nc.vector.tensor_scalar(K1[:], V[:], PI, None, ao.is_ge)
                    nc.vector.tensor_scalar(K2[:], V[:], 3 * PI, None, ao.is_ge)
                    nc.vector.tensor_scalar(K3[:], V[:], -PI, None, ao.is_le)
                    nc.vector.tensor_scalar(K4[:], V[:], -3 * PI, None, ao.is_le)
                    nc.vector.tensor_tensor(K1[:], K1[:], K2[:], ao.add)
                    nc.vector.tensor_tensor(K3[:], K3[:], K4[:], ao.add)
                    nc.vector.tensor_tensor(K1[:], K1[:], K3[:], ao.subtract)
                    nc.vector.scalar_tensor_tensor(
                        V[:], K1[:], -2 * PI, V[:], ao.mult, ao.add
                    )
                    sq_dst = planes["sq"][:, t * WPAD + 2 : t * WPAD + 514]
                    nc.vector.tensor_tensor(sq_dst, V[:], V[:], ao.mult)

                # ---- convs ----
                jcol = 0
                bias_col = 0
                p_hi_last = H - (S * (NT - 1) - HALO)  # 108
                for c in range(4):
                    dil, pad_top, pad_left, KH, KW = CONV_GEOM[c]
                    in_names = CONV_INPUTS[c]
                    O = CONV_OUT[c]
                    deltas = [kh * dil - pad_top for kh in range(KH)]
                    for o in range(O):
                        psums = [
                            psum_pool.tile([P, W], F32, tag="ps", name=f"ps_{c}_{o}_{t}")
                            for t in range(NT)
                        ]
                        for ci, nm in enumerate(in_names):
                            for kw in range(KW):
                                band = band_pool.tile([P, P], mm_dt, tag="band")
                                if skip_bands:
                                    deltas_eff = []
                                    jcol += len(deltas)
                                else:
                                    deltas_eff = deltas
                                for i, d in enumerate(deltas_eff):
                                    w_ap = wtab_sb[:, jcol : jcol + 1]
                                    jcol += 1
                                    src = ident_sb[
                                        :, (d + 2) * P : (d + 3) * P
                                    ]
                                    ao = mybir.AluOpType
                                    if i == 0:
                                        nc.vector.tensor_scalar(
                                            band[:], src, w_ap, None, ao.mult
                                        )
                                    else:
                                        nc.vector.scalar_tensor_tensor(
                                            band[:], src, w_ap, band[:], ao.mult, ao.add
                                        )
                                coff = 2 + kw * dil - pad_left
                                first = ci == 0 and kw == 0
                                last = ci == len(in_names) - 1 and kw == KW - 1
                                for t in range(NT):
                                    rhs = planes[nm][
                                        :, t * WPAD + coff : t * WPAD + coff + W
                                    ]
                                    nc.tensor.matmul(
                                        psums[t],
                                        (
                                            ident_sb[:, 2 * P : 3 * P].bitcast(mm_dt)
                                            if mm != "bf16"
                                            else ident_sb[:, 2 * P : 3 * P]
                                        )
                                        if skip_bands
                                        else band[:],
                                        rhs,
                                        start=first,
                                        stop=last,
                                    )
                        bias_ap = bias_sb[:, bias_col + o : bias_col + o + 1]
                        if c < 3:
                            out_nm = (
                                ["c1_0", "c1_1"][o]
                                if c == 0
                                else (f"c2_{o}" if c == 1 else f"c3_{o}")
                            )
                            for t in range(NT):
                                nc.scalar.add(
                                    pslice(out_nm, t, 2, 514), psums[t][:], bias_ap
                                )
                        else:
                            for t in range(NT):
                                st = c4_pool.tile([P, W], F32, tag="c4")
                                nc.scalar.add(st[:], psums[t][:], bias_ap)
                                rows = S if t < NT - 1 else H - S * (NT - 1)
                                nc.sync.dma_start(
                                    out=out_dram[o, S * t : S * t + rows, :],
                                    in_=st[HALO : HALO + rows, :],
                                )
                    # edge-zero the new planes (reference 'same' zero padding)
                    if c < 3:
                        outs = (
                            ["c1_0", "c1_1"]
                            if c == 0
                            else (
                                [f"c2_{i}" for i in range(4)]
                                if c == 1
                                else [f"c3_{i}" for i in range(8)]
                            )
                        )
                        for nm in outs:
                            nc.gpsimd.memset(msafe(planes[nm][0:HALO, 0:WPAD]), 0.0)
                            nc.gpsimd.memset(
                                msafe(
                                    planes[nm][
                                        p_hi_last:P, (NT - 1) * WPAD : NT * WPAD
                                    ]
                                ),
                                0.0,
                            )
                    bias_col += O

                # ---- remaining output channels from stored planes ----
                for ch in range(16, 48 if out_mode == 'full' else 16):
                    nm = CH_MAP[ch]
                    for t in range(NT):
                        rows = S if t < NT - 1 else H - S * (NT - 1)
                        src_ap = planes[nm][
                            HALO : HALO + rows, t * WPAD + 2 : t * WPAD + 514
                        ]
                        if mm == 'bf16':
                            nc.gpsimd.dma_start(
                                out=out_dram[ch, S * t : S * t + rows, :],
                                in_=src_ap,
                            )
                        else:
                            nc.sync.dma_start(
                                out=out_dram[ch, S * t : S * t + rows, :],
                                in_=src_ap.bitcast(F32),
                            )

    nc.compile()
    return nc


_NC_CACHE = None


def _get_nc():
    global _NC_CACHE
    if _NC_CACHE is None:
        _NC_CACHE = build_nc(mm=DEFAULT_MM)
    return _NC_CACHE


def _run(inputs, trace=False):
    inputs = {k: np.asarray(v) for k, v in inputs.items()}
    nc = _get_nc()
    wtab, ident, bias = _host_tables(inputs)
    feat = inputs["feature_in"].astype(np.float32)  # [8,1,512,512]
    n_cores = feat.shape[0]
    in_maps = [
        {"p": feat[b, 0], "ident": ident, "wtab": wtab, "bias": bias}
        for b in range(n_cores)
    ]
    res = bass_utils.run_bass_kernel_spmd(
        nc, in_maps, core_ids=list(range(n_cores)), trace=trace
    )
    out = np.stack([res.results[b]["out"] for b in range(n_cores)], axis=0)
    return out.astype(np.float32), res


def kernel(**inputs):
    return _run(inputs, trace=False)[0]



# revision 2
# speedup vs baseline: 2.7715x; 2.7715x over previous
"""Trainium2 Bass kernel for nn_Encoder_P — stacked-plane matmul architecture.

Per core (1 batch sample), bf16 on-chip, f32 PSUM accumulation:
  - H split into NW=19 windows of 32 rows, stride 28 (2-row halo each side).
  - 4 planes per 128-partition tile, row-major interleaved: partition
    p = 4*r + s (r = row in window, s = plane slot), so halo rows and
    per-slot reads are plain/strided partition slices (DMA-able):
      g0 = [sq, c1_0, c1_1, zero];  g1 = c2_0..3;  g2 = c3_0..3;  g3 = c3_4..7
  - Each conv = banded block matmuls: lhsT [128,128] (host-built) contracts
    4 planes x 32 rows at once, KW taps via PSUM accumulation over shifted
    rhs columns. conv1 reads the contiguous front-end sq scratch directly
    and its bands also pass sq through (identity block) so one ACT writes
    the whole g0 window.
  - Device emits the 31 unique planes bf16; host replicates the 17
    duplicated concat channels and upcasts.
"""

import numpy as np

import concourse.bacc as bacc
import concourse.bass as bass
import concourse.mybir as mybir
import concourse.tile as tile
from concourse import bass_utils

F32 = mybir.dt.float32
BF16 = mybir.dt.bfloat16
NPBF16 = mybir.dt.np(BF16)

H = 512
W = 512
NW = 19        # H windows
VS = 28        # valid rows per window (stride)
TW = 516       # 2 zero cols + 512 + 2 zero cols
TS = 112       # front-end tile stride (4 windows per tile)
P = 128
PI = float(np.pi)

# conv geometry: (dil, pad_top, pad_left, KH, KW)
CONV_GEOM = [
    (1, 1, 1, 4, 4),
    (2, 2, 2, 3, 3),
    (3, 1, 1, 2, 2),
    (4, 0, 0, 1, 1),
]

# band table layout: conv1 (j,kw) | conv2 kw | conv3 (mg,kg,kw) | conv4 (mg,kg)
NB1 = 16
NBAND = NB1 + 3 + 8 + 16

# K-group source maps: (kgrp, slot) per weight-ci index
C2_SRC = [(0, 1), (0, 2), (0, 0)]                      # c1_0, c1_1, sq
C3_SRC = [(1, 0), (1, 1), (1, 2), (1, 3), (0, 1), (0, 2), (0, 0)]
C4_SRC = ([(2, i) for i in range(4)] + [(3, i) for i in range(4)]
          + [(1, i) for i in range(4)] + [(0, 1), (0, 2), (0, 0)])

# final 48-channel map -> unique channel idx (device out order)
_UNIQ_IDX = {
    "sq": 0, "c1_0": 1, "c1_1": 2,
    **{f"c2_{i}": 3 + i for i in range(4)},
    **{f"c3_{i}": 7 + i for i in range(8)},
    **{f"c4_{i}": 15 + i for i in range(16)},
}
_CH_MAP = (
    [f"c4_{i}" for i in range(16)]
    + [f"c3_{i}" for i in range(8)]
    + [f"c2_{i}" for i in range(4)]
    + ["c1_0", "c1_1", "sq", "sq", "c1_0", "c1_1", "sq", "sq"]
    + [f"c2_{i}" for i in range(4)]
    + ["c1_0", "c1_1", "sq", "sq"]
    + ["c1_0", "c1_1", "sq", "sq"]
)
FINAL_IDX = np.array([_UNIQ_IDX[nm] for nm in _CH_MAP], np.int64)
N_UNIQ = 31


def _fold_weights(w1, w2, w3, w4):
    w3f = np.zeros((8, 7, 2, 2), np.float32)
    w3f[:, :6] = w3[:, :6]
    w3f[:, 6] = w3[:, 6] + w3[:, 7]
    w4f = np.zeros((16, 15, 1, 1), np.float32)
    w4f[:, :12] = w4[:, :12]
    w4f[:, 12] = w4[:, 12] + w4[:, 16]
    w4f[:, 13] = w4[:, 13] + w4[:, 17]
    w4f[:, 14] = w4[:, 14] + w4[:, 15] + w4[:, 18] + w4[:, 19]
    return w1.astype(np.float32), w2.astype(np.float32), w3f, w4f


def _host_tables(inputs):
    """bands [128, NBAND*128] (lhsT blocks), bias [128, 8], both interleaved
    partition layout p = 4*r + s."""
    w1, w2, w3f, w4f = _fold_weights(
        np.asarray(inputs["w1"]), np.asarray(inputs["w2"]),
        np.asarray(inputs["w3"]), np.asarray(inputs["w4"]))
    Bt = np.zeros((NBAND, P, P), np.float32)

    def badd(b, s_i, s_o, d, wgt):
        # interleaved: lhsT[4*(r_o+d)+s_i, 4*r_o+s_o] += w
        for r_o in range(32):
            r_i = r_o + d
            if 0 <= r_i < 32:
                Bt[b, 4 * r_i + s_i, 4 * r_o + s_o] += wgt

    # conv1: K = contiguous sq scratch rows (tile-local), M = window (4r+s)
    for j in range(4):
        for kw in range(4):
            b = j * 4 + kw
            for r_o in range(32):
                if kw == 1:  # identity pass-through of sq into slot 0
                    # (kw=1 is the unshifted column tap: coff = 2+1*1-1 = 2)
                    Bt[b, 28 * j + r_o, 4 * r_o + 0] = 1.0
                for o in range(2):
                    for kh in range(4):
                        k = 28 * j + r_o + kh - 1
                        if 0 <= k < P:
                            Bt[b, k, 4 * r_o + 1 + o] += w1[o, 0, kh, kw]
    bi = NB1
    for kw in range(3):                                   # conv2
        for o in range(4):
            for ci, (g, s) in enumerate(C2_SRC):
                for kh in range(3):
                    badd(bi, s, o, kh * 2 - 2, w2[o, ci, kh, kw])
        bi += 1
    for mg in range(2):                                   # conv3
        for kg in range(2):
            for kw in range(2):
                for oo in range(4):
                    for ci, (g, s) in enumerate(C3_SRC):
                        if g != kg:
                            continue
                        for kh in range(2):
                            badd(bi, s, oo, kh * 3 - 1,
                                 w3f[mg * 4 + oo, ci, kh, kw])
                bi += 1
    for mg in range(4):                                   # conv4
        for kg in range(4):
            for oo in range(4):
                for ci, (g, s) in enumerate(C4_SRC):
                    if g != kg:
                        continue
                    badd(bi, s, oo, 0, w4f[mg * 4 + oo, ci, 0, 0])
            bi += 1
    assert bi == NBAND
    bands = np.ascontiguousarray(
        Bt.transpose(1, 0, 2).reshape(P, NBAND * P)).astype(NPBF16)

    bias = np.zeros((P, 8), np.float32)
    b1, b2, b3, b4 = (np.asarray(inputs[k], np.float32)
                      for k in ("b1", "b2", "b3", "b4"))
    r = np.arange(32)
    for s in range(4):
        pr = 4 * r + s
        bias[pr, 0] = [0.0, b1[0], b1[1], 0.0][s]
        bias[pr, 1] = b2[s]
        bias[pr, 2] = b3[s]
        bias[pr, 3] = b3[4 + s]
        for mg in range(4):
            bias[pr, 4 + mg] = b4[mg * 4 + s]
    return bands, bias




def _assemble(rg, rc):
    """De-interleave raw dumps [4, 128, NW, W] into [31, H, W] planes."""
    nfull = NW - 1
    uniq = np.empty((N_UNIQ, H, W), np.float32)

    def put(v128, ch):
        v = v128.reshape(32, 4, NW, W)
        s = ch_s
        uniq[ch, :nfull * VS] = v[2:30, s, :nfull].transpose(
            1, 0, 2).reshape(nfull * VS, W)
        uniq[ch, nfull * VS:] = v[2:2 + H - nfull * VS, s, nfull]

    rg = np.asarray(rg).astype(np.float32)
    rc = np.asarray(rc).astype(np.float32)
    for g in range(4):
        for ch_s in range(4):
            ch = [0, 1, 2, None][ch_s] if g == 0 else 3 + (g - 1) * 4 + ch_s
            if ch is not None:
                put(rg[g], ch)
    for mg in range(4):
        for ch_s in range(4):
            put(rc[mg], 15 + mg * 4 + ch_s)
    return uniq



def build_nc(loop_k=1):
    nc = bacc.Bacc("TRN2", target_bir_lowering=False, debug=False)

    p_dram = nc.dram_tensor("p", [H, W], F32, kind="ExternalInput")
    bands_dram = nc.dram_tensor("bands", [P, NBAND * P], BF16,
                                kind="ExternalInput")
    bias_dram = nc.dram_tensor("bias", [P, 8], F32, kind="ExternalInput")
    outg_dram = nc.dram_tensor("outg", [4, P, NW, W], BF16,
                               kind="ExternalOutput")
    outc4_dram = nc.dram_tensor("outc4", [4, P, NW, W], BF16,
                                kind="ExternalOutput")

    groups = [nc.alloc_sbuf_tensor(f"g{g}", [P, NW * TW], BF16)
              for g in range(4)]
    c4t = [nc.alloc_sbuf_tensor(f"c4t{i}", [P, NW * W], BF16)
           for i in range(2)]
    zero_sb = nc.alloc_sbuf_tensor("zero_sb", [P, TW], BF16)
    band_sb = nc.alloc_sbuf_tensor("band_sb", [P, NBAND * P], BF16)
    bias_sb = nc.alloc_sbuf_tensor("bias_sb", [P, 8], F32)

    def gwin(g, w, c0, c1):
        return groups[g][:, w * TW + c0: w * TW + c1]

    ao = mybir.AluOpType

    with tile.TileContext(nc) as tc:
        with (
            tc.tile_pool(name="io", bufs=3) as io_pool,
            tc.tile_pool(name="fr", bufs=2) as fr_pool,
            tc.tile_pool(name="sq", bufs=2) as sq_pool,
            tc.tile_pool(name="psum", bufs=8, space="PSUM") as psum_pool,
        ):
            for _it in range(loop_k):
                nc.gpsimd.memset(zero_sb[:, :], 0.0)
                # ---- pad cols of plane groups ----
                for g in range(4):
                    gv = groups[g][:].rearrange("p (w c) -> p w c", w=NW)
                    nc.gpsimd.memset(gv[:, :, 0:2], 0.0)
                    nc.gpsimd.memset(gv[:, :, 514:516], 0.0)

                # ---- front-end tile: sq[p] for rows TS*t-2+p (516 wide) ----
                sq_tiles = {}

                def front_tile(t):
                    A = io_pool.tile([P, W], F32, tag="A")
                    Bf = io_pool.tile([P, W], F32, tag="B")
                    r0 = TS * t - 2   # row of partition 0
                    if t == 0:
                        nc.gpsimd.memset(A[0:2, :], 0.0)
                        nc.gpsimd.memset(Bf[0:3, :], 0.0)
                        nc.sync.dma_start(out=A[2:P, :], in_=p_dram[0:126, :])
                        nc.sync.dma_start(out=Bf[2:3, :], in_=p_dram[0:1, :])
                        nc.sync.dma_start(out=Bf[3:P, :], in_=p_dram[0:125, :])
                    elif r0 + P > H:
                        n = H - r0          # n = 66 < 128; zero the tail
                        nc.gpsimd.memset(A[64:P, :], 0.0)
                        nc.gpsimd.memset(Bf[64:P, :], 0.0)
                        nc.sync.dma_start(out=A[0:n, :], in_=p_dram[r0:H, :])
                        nc.sync.dma_start(out=Bf[0:n, :],
                                          in_=p_dram[r0 - 1:H - 1, :])
                    else:
                        nc.sync.dma_start(out=A[:, :], in_=p_dram[r0:r0 + P, :])
                        nc.sync.dma_start(out=Bf[:, :],
                                          in_=p_dram[r0 - 1:r0 + P - 1, :])
                    V = fr_pool.tile([P, W], F32, tag="V")
                    K1 = fr_pool.tile([P, W], F32, tag="K1")
                    K2 = fr_pool.tile([P, W], F32, tag="K2")
                    SQ = sq_pool.tile([P, TW], BF16, tag="SQ")
                    nc.vector.tensor_tensor(V[:], A[:], Bf[:], ao.subtract)
                    # wrap(v) = v - 2*pi*k, k = (v>=pi)+(v>=3pi)-(v<=-pi)-(v<=-3pi)
                    nc.vector.tensor_scalar(K1[:], V[:], PI, None, ao.is_ge)
                    nc.vector.tensor_scalar(K2[:], V[:], 3 * PI, None, ao.is_ge)
                    nc.vector.tensor_tensor(K1[:], K1[:], K2[:], ao.add)
                    nc.vector.tensor_scalar(K2[:], V[:], -PI, None, ao.is_le)
                    nc.vector.tensor_tensor(K1[:], K1[:], K2[:], ao.subtract)
                    nc.vector.tensor_scalar(K2[:], V[:], -3 * PI, None, ao.is_le)
                    nc.vector.tensor_tensor(K1[:], K1[:], K2[:], ao.subtract)
                    nc.vector.scalar_tensor_tensor(
                        V[:], K1[:], -2 * PI, V[:], ao.mult, ao.add)
                    nc.vector.tensor_tensor(SQ[:, 2:514], V[:], V[:], ao.mult)
                    nc.gpsimd.memset(SQ[:, 0:2], 0.0)
                    nc.gpsimd.memset(SQ[:, 514:516], 0.0)
                    sq_tiles[t] = SQ

                # ---- halo resolution (interleaved: 1 DMA per group/dir) ----
                def halo_pairs(g, lo, hi):
                    gv = groups[g][:].rearrange("p (w c) -> p w c", w=NW)
                    nc.sync.dma_start(      # rows 28,29 of w -> 0,1 of w+1
                        out=gv[0:8, lo + 1:hi + 1, 2:514],
                        in_=gv[112:120, lo:hi, 2:514],
                    )
                    nc.sync.dma_start(      # rows 2,3 of w+1 -> 30,31 of w
                        out=gv[120:128, lo:hi, 2:514],
                        in_=gv[8:16, lo + 1:hi + 1, 2:514],
                    )

                def halo_chunk(q, gs):
                    for g in gs:
                        if q == 1:
                            nc.gpsimd.memset(groups[g][0:8, 0:TW], 0.0)
                            halo_pairs(g, 0, 5)
                        elif q == 3:
                            halo_pairs(g, 5, 13)
                        elif q == 4:
                            halo_pairs(g, 13, 18)
                            # rows >= 512 of window 18 -> zero (DMA: start
                            # partition 40 is not memset-able)
                            nc.sync.dma_start(
                                out=groups[g][40:P, (NW - 1) * TW:NW * TW],
                                in_=zero_sb[40:P, :])

                def mm(psum, bi, rhs, first, last):
                    nc.tensor.matmul(
                        psum, band_sb[:, bi * P:(bi + 1) * P], rhs,
                        start=first, stop=last)

                # ---- conv1 quad (windows 4t..4t+4 from sq tile t) ----
                def conv1_quad(t):
                    ws = range(4 * t, min(4 * t + 4, NW))
                    SQ = sq_tiles[t]
                    pss = {w: psum_pool.tile([P, W], F32, tag="ps",
                                             name=f"c1_{w}") for w in ws}
                    for kw in range(4):
                        for w in ws:
                            j = w - 4 * t
                            mm(pss[w], j * 4 + kw,
                               SQ[:, 1 + kw: 1 + kw + W], kw == 0, kw == 3)
                    for w in ws:
                        nc.scalar.add(gwin(0, w, 2, 514), pss[w][:],
                                      bias_sb[:, 0:1])

                # ---- generic conv quad over group inputs ----
                def conv_quad(q, taps, act_fn):
                    ws = range(4 * q, min(4 * q + 4, NW))
                    pss = {w: psum_pool.tile([P, W], F32, tag="ps",
                                             name=f"cx_{taps[0][0]}_{w}")
                           for w in ws}
                    for n, (bi, kg, coff) in enumerate(taps):
                        for w in ws:
                            mm(pss[w], bi,
                               groups[kg][:, w * TW + coff: w * TW + coff + W],
                               n == 0, n == len(taps) - 1)
                    for w in ws:
                        act_fn(w, pss[w])

                def dump_g(g, w0, w1):
                    gv = groups[g][:].rearrange("p (w c) -> p w c", w=NW)
                    nc.sync.dma_start(out=outg_dram[g, :, w0:w1, :],
                                      in_=gv[:, w0:w1, 2:514])

                def dump_c4(mg, w0, w1):
                    cv = c4t[mg % 2][:].rearrange("p (w c) -> p w c", w=NW)
                    nc.sync.dma_start(out=outc4_dram[mg, :, w0:w1, :],
                                      in_=cv[:, w0:w1, :])

                d2, pt2, pl2, KH2, KW2 = CONV_GEOM[1]
                taps2 = [(NB1 + kw, 0, 2 + kw * d2 - pl2) for kw in range(KW2)]
                act2 = lambda w, ps: nc.scalar.add(
                    gwin(1, w, 2, 514), ps[:], bias_sb[:, 1:2])
                d3, pt3, pl3, KH3, KW3 = CONV_GEOM[2]
                taps3 = [
                    [(NB1 + 3 + mg * 4 + kg * 2 + kw, kg, 2 + kw * d3 - pl3)
                     for kg in range(2) for kw in range(KW3)]
                    for mg in range(2)
                ]
                def act3(mg):
                    return lambda w, ps: nc.scalar.add(
                        gwin(2 + mg, w, 2, 514), ps[:],
                        bias_sb[:, 2 + mg:3 + mg])
                taps4 = [[(NB1 + 11 + mg * 4 + kg, kg, 2) for kg in range(4)]
                         for mg in range(4)]
                def act4(mg):
                    return lambda w, ps: nc.scalar.add(
                        c4t[mg % 2][:, w * W:(w + 1) * W], ps[:],
                        bias_sb[:, 4 + mg:5 + mg])

                # conv1 interleaved with front-end tiles (tile t == quad t)
                for t in range(5):
                    front_tile(t)
                    if t == 0:  # params off the input loads' critical path
                        nc.sync.dma_start(out=band_sb[:, 0:NB1 * P],
                                          in_=bands_dram[:, 0:NB1 * P])
                        nc.sync.dma_start(out=bias_sb[:], in_=bias_dram[:])
                        nc.sync.dma_start(out=band_sb[:, NB1 * P:],
                                          in_=bands_dram[:, NB1 * P:])
                    conv1_quad(t)
                    halo_chunk(t, [0])

                for q in range(5):
                    conv_quad(q, taps2, act2)
                    halo_chunk(q, [1])

                for q in range(5):
                    for mg in range(2):
                        conv_quad(q, taps3[mg], act3(mg))
                    halo_chunk(q, [2, 3])

                for q in range(5):
                    w0, w1 = 4 * q, min(4 * q + 4, NW)
                    for mg in range(4):
                        conv_quad(q, taps4[mg], act4(mg))
                        dump_c4(mg, w0, w1)
                    for g in range(4):
                        dump_g(g, w0, w1)

    nc.compile()
    return nc


_NC_CACHE = {}


def _get_nc(loop_k=1):
    if loop_k not in _NC_CACHE:
        _NC_CACHE[loop_k] = build_nc(loop_k=loop_k)
    return _NC_CACHE[loop_k]


def _run(inputs, trace=False):
    inputs = {k: np.asarray(v) for k, v in inputs.items()}
    nc = _get_nc()
    bands, bias = _host_tables(inputs)
    feat = inputs["feature_in"].astype(np.float32)
    n_cores = feat.shape[0]
    in_maps = [
        {"p": feat[b, 0], "bands": bands, "bias": bias}
        for b in range(n_cores)
    ]
    res = bass_utils.run_bass_kernel_spmd(
        nc, in_maps, core_ids=list(range(n_cores)), trace=trace
    )
    out = np.empty((n_cores, 48, H, W), np.float32)
    for b in range(n_cores):
        uniq = _assemble(res.results[b]["outg"], res.results[b]["outc4"])
        out[b] = uniq[FINAL_IDX]
    return out, res


def kernel(**inputs):
    return _run(inputs, trace=False)[0]
